# revision 16
# baseline (speedup 1.0000x reference)
"""Fused AllReduce + residual-add + RMSNorm kernel for one TRN2 chip (8 NeuronCores).

Reference computation (for full input [tp=8, tokens=4096, hidden=4096] f32):
    reduced = input.sum(axis=0)
    hidden  = reduced + residual
    norm    = hidden * rsqrt(mean(hidden^2, -1) + 1e-6) * norm_weight
    return (norm, hidden)

Sharding strategy: shard the TOKEN axis, not the tp axis. Core c receives
input[:, c*512:(c+1)*512, :] -- all 8 partial sums for its 512 tokens -- and
does a purely local 8-way sum + residual + RMSNorm. No collective needed,
perfect parallelism, and total HBM traffic equals the unavoidable minimum.

Per-core pipeline (4 token-tiles of 128 tokens x 4096 hidden):
  - DMA: residual tile + 4x 2-slab input groups (4MB transfers).
  - TensorE: 9 identity-matmuls per PSUM bank accumulate res + 8 slabs into
    PSUM (float32r -> 1 cycle/row). Vector engine stays nearly free.
  - ScalarE: copy PSUM->SBUF (hidden tile).
  - VectorE: bn_stats/bn_aggr for mean(h^2), rstd, and the two norm muls.
  - DMA out: hidden + norm tiles.
"""

import numpy as np

import concourse.bass as bass
import concourse.tile as tile
from concourse import bacc, mybir
from concourse.bass_utils import run_bass_kernel_spmd
TP = 8
TOKENS = 4096
HIDDEN = 4096
N_CORES = 8
TOK_PER_CORE = TOKENS // N_CORES  # 512
P = 128  # SBUF partitions
N_TILES = TOK_PER_CORE // P  # 4 token-tiles per core
EPS = 1e-6
F32 = mybir.dt.float32
F32R = mybir.dt.float32r
BN_F = 512  # bn_stats max free size
NB = HIDDEN // 512  # PSUM banks per tile (8)
GRP = 2  # input slabs per DMA group


def _build():
    nc = bacc.Bacc("TRN2")
    x_ext = nc.declare_dram_parameter(
        "input", [TP, TOK_PER_CORE, HIDDEN], F32R, isOutput=False
    )
    r_ext = nc.declare_dram_parameter(
        "residual", [TOK_PER_CORE, HIDDEN], F32R, isOutput=False
    )
    w_ext = nc.declare_dram_parameter("norm_weight", [HIDDEN], F32, isOutput=False)
    norm_ext = nc.declare_dram_parameter(
        "norm", [TOK_PER_CORE, HIDDEN], F32, isOutput=True
    )
    hid_ext = nc.declare_dram_parameter(
        "hidden", [TOK_PER_CORE, HIDDEN], F32, isOutput=True
    )
    id_ext = nc.declare_dram_parameter("ident", [P, P], F32R, isOutput=False)

    with tile.TileContext(nc) as tc:
        with (
            tc.tile_pool(name="singles", bufs=1) as singles,
            tc.tile_pool(name="xsp", bufs=3) as xsp,
            tc.tile_pool(name="resp", bufs=2) as resp,
            tc.tile_pool(name="hidp", bufs=2) as hidp,
            tc.tile_pool(name="normp", bufs=1) as normp,
            tc.tile_pool(name="statsp", bufs=2) as statsp,
            tc.tile_pool(name="psump", bufs=1, space="PSUM") as psump,
        ):
            ident = None
            w_b = None
            eps_t = None

            for it in range(N_TILES):
                t0 = it * P
                res_t = resp.tile([P, HIDDEN], F32R, tag="res")
                nc.scalar.dma_start(out=res_t, in_=r_ext[t0 : t0 + P, :])
                xs_tiles = []
                for g in range(TP // GRP):
                    xs = xsp.tile([P, GRP, HIDDEN], F32R, tag="xs")
                    src = x_ext[g * GRP : (g + 1) * GRP, t0 : t0 + P, :].rearrange(
                        "p t h -> t p h"
                    )
                    eng = nc.sync if g % 2 == 0 else nc.scalar
                    eng.dma_start(out=xs, in_=src)
                    xs_tiles.append(xs)

                if it == 0:
                    # one-time setup, issued after the first input loads so the
                    # DMA pipeline starts on the critical-path bytes
                    ident = singles.tile([P, P], F32R)
                    nc.sync.dma_start(out=ident, in_=id_ext[:, :])
                    w_b = singles.tile([P, HIDDEN], F32)
                    w_ap = w_ext[:]
                    w_bcast = bass.AP(
                        tensor=w_ap.tensor, offset=w_ap.offset, ap=[[0, P], w_ap.ap[0]]
                    )
                    nc.gpsimd.dma_start(out=w_b, in_=w_bcast)
                    eps_t = singles.tile([P, 1], F32)
                    nc.vector.memset(eps_t, EPS)

                # PSUM accumulate: res + 8 slabs, via identity matmul (f32r)
                psum_t = psump.tile([P, HIDDEN], F32, tag="ps")
                for b in range(NB):
                    nc.tensor.matmul(
                        psum_t[:, b * 512 : (b + 1) * 512],
                        ident,
                        res_t[:, b * 512 : (b + 1) * 512],
                        start=True,
                        stop=False,
                    )
                for g in range(TP // GRP):
                    for j in range(GRP):
                        last = g == TP // GRP - 1 and j == GRP - 1
                        for b in range(NB):
                            nc.tensor.matmul(
                                psum_t[:, b * 512 : (b + 1) * 512],
                                ident,
                                xs_tiles[g][:, j, b * 512 : (b + 1) * 512],
                                start=False,
                                stop=last,
                            )

                # hidden = PSUM -> SBUF via ScalarE, then DMA out
                hid_t = hidp.tile([P, HIDDEN], F32, tag="hid")
                nc.scalar.copy(out=hid_t, in_=psum_t)
                nc.scalar.dma_start(out=hid_ext[t0 : t0 + P, :], in_=hid_t)

                # row stats: mean(h^2) = var + mean^2 via bn_stats/bn_aggr
                sts = statsp.tile([P, NB, 6], F32, tag="sts")
                resh = hid_t.rearrange("p (n f) -> p n f", f=BN_F)
                for i in range(NB):
                    nc.vector.bn_stats(out=sts[:, i, :], in_=resh[:, i, :])
                mv = statsp.tile([P, 2], F32, tag="mv")
                nc.vector.bn_aggr(out=mv, in_=sts)
                msq = statsp.tile([P, 1], F32, tag="msq")
                nc.vector.tensor_mul(out=msq, in0=mv[:, 0:1], in1=mv[:, 0:1])
                nc.vector.tensor_add(out=msq, in0=msq, in1=mv[:, 1:2])
                rstd = statsp.tile([P, 1], F32, tag="rstd")
                nc.scalar.activation(
                    out=rstd,
                    in_=msq,
                    func=mybir.ActivationFunctionType.Sqrt,
                    bias=eps_t,
                    scale=1.0,
                )
                nc.vector.reciprocal(out=rstd, in_=rstd)

                # norm = hidden * rstd * w
                nt = normp.tile([P, HIDDEN], F32, tag="nt")
                nc.vector.tensor_scalar_mul(out=nt, in0=hid_t, scalar1=rstd)
                nc.vector.tensor_mul(out=nt, in0=nt, in1=w_b)
                nc.scalar.dma_start(out=norm_ext[t0 : t0 + P, :], in_=nt)

    nc.finalize()  # Bacc: runs compile passes (event-sem split, reg alloc)
    return nc


_NC = None


def _get_nc():
    global _NC
    if _NC is None:
        _NC = _build()
    return _NC


def _run(input, residual, norm_weight, trace=False):
    input = np.ascontiguousarray(np.asarray(input), dtype=np.float32)
    residual = np.ascontiguousarray(np.asarray(residual), dtype=np.float32)
    norm_weight = np.ascontiguousarray(np.asarray(norm_weight), dtype=np.float32)

    in_maps = []
    for c in range(N_CORES):
        t0 = c * TOK_PER_CORE
        in_maps.append(
            {
                "input": np.ascontiguousarray(input[:, t0 : t0 + TOK_PER_CORE, :]),
                "residual": np.ascontiguousarray(residual[t0 : t0 + TOK_PER_CORE, :]),
                "norm_weight": norm_weight,
                "ident": np.eye(P, dtype=np.float32),
            }
        )
    res = run_bass_kernel_spmd(
        _get_nc(), in_maps, core_ids=list(range(N_CORES)), trace=trace
    )
    outs = res.results
    norm = np.concatenate([outs[c]["norm"] for c in range(N_CORES)], axis=0)
    hidden = np.concatenate([outs[c]["hidden"] for c in range(N_CORES)], axis=0)
    return (norm, hidden), res


def kernel(input, residual, norm_weight):
    (norm, hidden), _ = _run(input, residual, norm_weight, trace=False)
    return norm, hidden


# revision 17
# speedup vs baseline: 1.1905x; 1.1905x over previous
"""Fused AllReduce + residual-add + RMSNorm kernel for one TRN2 chip (8 NeuronCores).

Reference computation (for full input [tp=8, tokens=4096, hidden=4096] f32):
    reduced = input.sum(axis=0)
    hidden  = reduced + residual
    norm    = hidden * rsqrt(mean(hidden^2, -1) + 1e-6) * norm_weight
    return (norm, hidden)

Sharding strategy: shard the TOKEN axis, not the tp axis. Core c receives
input[:, c*512:(c+1)*512, :] -- all 8 partial sums for its 512 tokens -- and
does a purely local 8-way sum + residual + RMSNorm. No collective needed,
perfect parallelism, and total HBM traffic equals the unavoidable minimum.

Per-core pipeline (4 token-tiles of 128 tokens x 4096 hidden):
  - DMA: residual tile + 4x 2-slab input groups (4MB transfers).
  - TensorE: 9 identity-matmuls per PSUM bank accumulate res + 8 slabs into
    PSUM (float32r -> 1 cycle/row). Vector engine stays nearly free.
  - ScalarE: copy PSUM->SBUF (hidden tile).
  - VectorE: bn_stats/bn_aggr for mean(h^2), rstd, and the two norm muls.
  - DMA out: hidden + norm tiles.
"""

import numpy as np

import concourse.bass as bass
import concourse.tile as tile
from concourse import bacc, mybir
from concourse.bass_utils import run_bass_kernel_spmd
TP = 8
TOKENS = 4096
HIDDEN = 4096
N_CORES = 8
TOK_PER_CORE = TOKENS // N_CORES  # 512
P = 128  # SBUF partitions
N_TILES = TOK_PER_CORE // P  # 4 token-tiles per core
EPS = 1e-6
F32 = mybir.dt.float32
F32R = mybir.dt.float32r
BN_F = 512  # bn_stats max free size
NB = HIDDEN // 512  # PSUM banks per tile (8)
GRP = 2  # input slabs per DMA group


def _build():
    nc = bacc.Bacc("TRN2")
    x_ext = nc.declare_dram_parameter(
        "input", [TP, TOK_PER_CORE, HIDDEN], F32R, isOutput=False
    )
    r_ext = nc.declare_dram_parameter(
        "residual", [TOK_PER_CORE, HIDDEN], F32R, isOutput=False
    )
    w_ext = nc.declare_dram_parameter("norm_weight", [HIDDEN], F32, isOutput=False)
    norm_ext = nc.declare_dram_parameter(
        "norm", [TOK_PER_CORE, HIDDEN], F32, isOutput=True
    )
    hid_ext = nc.declare_dram_parameter(
        "hidden", [TOK_PER_CORE, HIDDEN], F32, isOutput=True
    )
    id_ext = nc.declare_dram_parameter("ident", [P, P], F32R, isOutput=False)

    with tile.TileContext(nc) as tc:
        with (
            tc.tile_pool(name="singles", bufs=1) as singles,
            tc.tile_pool(name="xsp", bufs=3) as xsp,
            tc.tile_pool(name="resp", bufs=2) as resp,
            tc.tile_pool(name="hidp", bufs=2) as hidp,
            tc.tile_pool(name="normp", bufs=1) as normp,
            tc.tile_pool(name="statsp", bufs=2) as statsp,
            tc.tile_pool(name="psump", bufs=1, space="PSUM") as psump,
        ):
            ident = singles.tile([P, P], F32R)
            nc.sync.dma_start(out=ident, in_=id_ext[:, :])

            # norm_weight broadcast to all 128 partitions (one-time 2MB DMA)
            w_b = singles.tile([P, HIDDEN], F32)
            w_ap = w_ext[:]
            w_bcast = bass.AP(
                tensor=w_ap.tensor, offset=w_ap.offset, ap=[[0, P], w_ap.ap[0]]
            )
            nc.gpsimd.dma_start(out=w_b, in_=w_bcast)
            eps_t = singles.tile([P, 1], F32)
            nc.vector.memset(eps_t, EPS)

            for it in range(N_TILES):
                t0 = it * P
                res_t = resp.tile([P, HIDDEN], F32R, tag="res")
                nc.scalar.dma_start(out=res_t, in_=r_ext[t0 : t0 + P, :])
                xs_tiles = []
                for g in range(TP // GRP):
                    xs = xsp.tile([P, GRP, HIDDEN], F32R, tag="xs")
                    src = x_ext[g * GRP : (g + 1) * GRP, t0 : t0 + P, :].rearrange(
                        "p t h -> t p h"
                    )
                    nc.sync.dma_start(out=xs, in_=src)
                    xs_tiles.append(xs)

                # PSUM accumulate: res + 8 slabs, via identity matmul (f32r)
                psum_t = psump.tile([P, HIDDEN], F32, tag="ps")
                for b in range(NB):
                    nc.tensor.matmul(
                        psum_t[:, b * 512 : (b + 1) * 512],
                        ident,
                        res_t[:, b * 512 : (b + 1) * 512],
                        start=True,
                        stop=False,
                    )
                for g in range(TP // GRP):
                    for j in range(GRP):
                        last = g == TP // GRP - 1 and j == GRP - 1
                        for b in range(NB):
                            nc.tensor.matmul(
                                psum_t[:, b * 512 : (b + 1) * 512],
                                ident,
                                xs_tiles[g][:, j, b * 512 : (b + 1) * 512],
                                start=False,
                                stop=last,
                            )

                # hidden = PSUM -> SBUF via ScalarE, then DMA out
                hid_t = hidp.tile([P, HIDDEN], F32, tag="hid")
                nc.scalar.copy(out=hid_t, in_=psum_t)
                nc.scalar.dma_start(out=hid_ext[t0 : t0 + P, :], in_=hid_t)

                # row stats: mean(h^2) = var + mean^2 via bn_stats/bn_aggr
                sts = statsp.tile([P, NB, 6], F32, tag="sts")
                resh = hid_t.rearrange("p (n f) -> p n f", f=BN_F)
                for i in range(NB):
                    nc.vector.bn_stats(out=sts[:, i, :], in_=resh[:, i, :])
                mv = statsp.tile([P, 2], F32, tag="mv")
                nc.vector.bn_aggr(out=mv, in_=sts)
                msq = statsp.tile([P, 1], F32, tag="msq")
                nc.vector.tensor_mul(out=msq, in0=mv[:, 0:1], in1=mv[:, 0:1])
                nc.vector.tensor_add(out=msq, in0=msq, in1=mv[:, 1:2])
                rstd = statsp.tile([P, 1], F32, tag="rstd")
                nc.scalar.activation(
                    out=rstd,
                    in_=msq,
                    func=mybir.ActivationFunctionType.Sqrt,
                    bias=eps_t,
                    scale=1.0,
                )
                nc.vector.reciprocal(out=rstd, in_=rstd)

                # norm = hidden * rstd * w
                nt = normp.tile([P, HIDDEN], F32, tag="nt")
                nc.vector.tensor_scalar_mul(out=nt, in0=hid_t, scalar1=rstd)
                nc.vector.tensor_mul(out=nt, in0=nt, in1=w_b)
                nc.scalar.dma_start(out=norm_ext[t0 : t0 + P, :], in_=nt)

    nc.finalize()  # Bacc: runs compile passes (event-sem split, reg alloc)
    return nc


_NC = None


def _get_nc():
    global _NC
    if _NC is None:
        _NC = _build()
    return _NC


def _run(input, residual, norm_weight, trace=False):
    input = np.ascontiguousarray(np.asarray(input), dtype=np.float32)
    residual = np.ascontiguousarray(np.asarray(residual), dtype=np.float32)
    norm_weight = np.ascontiguousarray(np.asarray(norm_weight), dtype=np.float32)

    in_maps = []
    for c in range(N_CORES):
        t0 = c * TOK_PER_CORE
        in_maps.append(
            {
                "input": np.ascontiguousarray(input[:, t0 : t0 + TOK_PER_CORE, :]),
                "residual": np.ascontiguousarray(residual[t0 : t0 + TOK_PER_CORE, :]),
                "norm_weight": norm_weight,
                "ident": np.eye(P, dtype=np.float32),
            }
        )
    res = run_bass_kernel_spmd(
        _get_nc(), in_maps, core_ids=list(range(N_CORES)), trace=trace
    )
    outs = res.results
    norm = np.concatenate([outs[c]["norm"] for c in range(N_CORES)], axis=0)
    hidden = np.concatenate([outs[c]["hidden"] for c in range(N_CORES)], axis=0)
    return (norm, hidden), res


def kernel(input, residual, norm_weight):
    (norm, hidden), _ = _run(input, residual, norm_weight, trace=False)
    return norm, hidden
